# revision 2
# baseline (speedup 1.0000x reference)
"""Multi-head graph attention (GAT-style message passing) on 8 Trainium2 cores.

Math (per head i, diag transform):
    h        = x * w[i]                      # [N, d]
    p_src    = h @ a[:d],  p_dst = h @ a[d:] # [N]
    s_e      = p_src[src_e] + p_dst[dst_e]   # per edge
    e_e      = exp(-leaky_relu(s_e, 0.2))
    out[i,n] = (sum_{e: src=n} e_e * h[dst_e]) / (sum_{e: src=n} e_e)

w[i] (diagonal) commutes with the segment sum, so
    out[i,n] = w[i] * (sum e_e * x[dst_e]) / rowsum_n
and the gather of x[dst] is shared by all 4 heads.

Design notes:
  - Gathers via dma_gather (one instruction per ~6k edges instead of one
    indirect DMA per 128): indices are int16 SIGNED offsets from the in_ap
    base (Q7 sign-extends); the node table is split into 2 chunks with
    mid-chunk bases, and rows at base-1 are dummies so no real index is -1
    (the HW strips trailing negative indices, so every gather column is
    sorted ascending-by-index to end nonnegative).
  - Each dma_gather's descriptors must fit the SWDGE carveout ring
    (scratch_size/64 descs per engine; descs = num_idxs/16+1), so
    dynamic_dma_scratch_size=65536 and gathers of K*SPG*128 = 6144 idxs.
  - Two SWDGE queues: queue q runs on Q7 core pair q, so the two chunks'
    gathers generate descriptors in parallel.
  - A supertile = window of <= W src nodes whose per-chunk edge lists each
    fit a 128-edge column; the 2 chunk-columns accumulate into one PSUM
    slot range (matmul start/stop chaining), keeping node slots compact
    and the device-side rowsum/divide exact.
  - p_src/p_dst per edge shipped as dense host metadata (host-gathered
    from the device-computed P table): direct score path.
  - w folded into the rowsum-broadcast selector; single fused epilogue
    multiply; fp16 output; one output DMA per group.
"""

import os

import numpy as np

from concourse import bacc, bass, mybir
import concourse.tile as tile
from concourse.bass_utils import run_bass_kernel_spmd

LAST_RESULTS = []

F32 = mybir.dt.float32
F16 = mybir.dt.float16
I16 = mybir.dt.int16

N_CORES = 8
TILE_E = 128     # edges per column (partition dim)
D = 128

NCHUNK = 2       # chunks of the gather table (signed-index mode)
W = 16           # max node span of a supertile
SPG = 16         # supertiles per group  (cols/group = NCHUNK*SPG = 32)
CG = NCHUNK * SPG
K = 3            # groups per supergroup (one gather per (supergroup, chunk))
NIDXC = K * SPG * TILE_E   # gather indices per (supergroup, chunk) = 6144
SCRATCH = 65536  # SWDGE ring: 1024 descs/engine; gather needs 6144/16+1
NQ = 2           # SWDGE queues (parallel Q7 descriptor generation)

N_NODES = 100000
HALF = N_NODES // 2
B0 = 25001       # gather bases (table-row space); rows 25000/75001 are
B1 = 75002       # dummies so no real local index is exactly -1
TAB_ROWS = N_NODES + 2


def _node_to_row(n):
    return n + (n >= 25000) + (n >= 75000)


def _node_chunk_idx(n):
    row = _node_to_row(n)
    ch = (n >= HALF).astype(np.int64)
    idx = row - np.where(ch == 0, B0, B1)
    return ch, idx.astype(np.int16)


# --------------------------------------------------------------------------
# host-side layout preprocessing
# --------------------------------------------------------------------------

def _prep_edges(src, dst, n_nodes, ptab):
    """Sort by (src, chunk), pack supertiles, build per-core arrays."""
    ch_all, idx_all = _node_chunk_idx(dst)
    order = np.lexsort((ch_all, src))
    src_s = src[order].astype(np.int64)
    ch_s = ch_all[order].astype(np.int8)
    gi_s = idx_all[order]
    p_src_e = ptab[src_s, 0:4].astype(np.float16)
    p_dst_e = ptab[dst[order], 4:8].astype(np.float16)

    npc = n_nodes // N_CORES
    core_bounds = np.searchsorted(src_s, np.arange(N_CORES + 1) * npc)

    # per-(node, chunk) edge segment starts within the lexsorted arrays
    per_core = []
    nsuper_max = 0
    for c in range(N_CORES):
        lo, hi = core_bounds[c], core_bounds[c + 1]
        s = src_s[lo:hi] - c * npc
        key = s * NCHUNK + ch_s[lo:hi]
        cnt = np.bincount(key, minlength=npc * NCHUNK)
        starts = lo + np.concatenate([[0], np.cumsum(cnt)[:-1]])
        cnt = cnt.reshape(npc, NCHUNK)
        starts = starts.reshape(npc, NCHUNK)
        # bin-pack nodes (arbitrary sets; loc = rank in the bin's node list)
        # into supertiles: per-chunk edge counts <= 128, <= W nodes.
        # natural src order beats degree-sorted here: the 16-node cap
        # strands small nodes when big ones go first
        bins = []          # (rem[NCHUNK], node_list)
        open_bins = []     # indices into bins
        for ni in range(npc):
            d = cnt[ni]
            placed = False
            for bi in open_bins:
                rem, lst = bins[bi]
                if len(lst) < W and (d <= rem).all():
                    rem -= d
                    lst.append(ni)
                    if len(lst) == W or rem.min() == 0:
                        open_bins.remove(bi)
                    placed = True
                    break
            if not placed:
                bins.append((np.array([TILE_E] * NCHUNK) - d, [ni]))
                open_bins.append(len(bins) - 1)
                if len(open_bins) > 48:
                    open_bins.pop(0)
        supers = [(sorted(lst), cnt, starts) for _, lst in bins]
        per_core.append(supers)
        nsuper_max = max(nsuper_max, len(supers))

    G = -(-nsuper_max // SPG)
    SG = -(-G // K)

    idxc_l, ps8_l, loc_l, colmap_l = [], [], [], []
    for c in range(N_CORES):
        supers = per_core[c]
        nsup = G * SPG
        gidx = np.zeros((nsup, NCHUNK, TILE_E), dtype=np.int16)
        p_arr = np.zeros((nsup, NCHUNK, TILE_E, 8), dtype=np.float16)
        l_arr = np.full((nsup, NCHUNK, TILE_E), -1.0, dtype=np.float16)
        cmap = np.full((nsup, W), -1, dtype=np.int64)
        for si, (lst, cnt, starts) in enumerate(supers):
            for cc in range(NCHUNK):
                sel = np.concatenate(
                    [np.arange(starts[ni, cc], starts[ni, cc] + cnt[ni, cc])
                     for ni in lst])
                ne = len(sel)
                if ne == 0:
                    continue
                loc_e = np.concatenate(
                    [np.full(cnt[ni, cc], r) for r, ni in enumerate(lst)]
                ).astype(np.float16)
                # ascending index order: the column (and so any gather run
                # ending on it) ends with its LARGEST index; pad slots are 0
                o = np.argsort(gi_s[sel], kind="stable")
                gidx[si, cc, :ne] = gi_s[sel][o]
                p_arr[si, cc, :ne, 0:4] = p_src_e[sel][o]
                p_arr[si, cc, :ne, 4:8] = p_dst_e[sel][o]
                l_arr[si, cc, :ne] = loc_e[o]
                assert ne < TILE_E or gidx[si, cc, TILE_E - 1] >= 0, \
                    "full column with all-negative indices"
            cmap[si, :len(lst)] = np.asarray(lst, dtype=np.int64) + c * npc
        # gather order: (sg, chunk, gl, s, p); wrapped into 16 partitions
        # (idx i at [i%16, i//16]) and replicated x8 down the partitions
        NIDX16 = NIDXC // 16
        idxc = np.zeros((SG, 128, NCHUNK * NIDX16), dtype=np.int16)
        for sg in range(SG):
            kk = min(K, G - sg * K)
            si0 = sg * K * SPG
            for cc in range(NCHUNK):
                flat = gidx[si0:si0 + kk * SPG, cc, :].reshape(-1)
                wrap = flat.reshape(-1, 16).T  # [16, n/16]
                for rep in range(8):
                    idxc[sg, rep * 16:(rep + 1) * 16,
                         cc * NIDX16:cc * NIDX16 + wrap.shape[1]] = wrap
        idxc_l.append(np.ascontiguousarray(idxc))
        # compute order: [G, 128, CG, ...] with col = cc*SPG + s
        p5 = p_arr.reshape(G, SPG, NCHUNK, TILE_E, 8).transpose(0, 3, 2, 1, 4)
        ps8_l.append(np.ascontiguousarray(p5.reshape(G, TILE_E, CG, 8)))
        l5 = l_arr.reshape(G, SPG, NCHUNK, TILE_E).transpose(0, 3, 2, 1)
        loc_l.append(np.ascontiguousarray(l5.reshape(G, TILE_E, CG)))
        colmap_l.append(cmap.reshape(G * SPG * W))
    return dict(idxc=idxc_l, ps8=ps8_l, loc=loc_l, colmap=colmap_l, G=G, SG=SG)


def _build_table(x16):
    tab = np.zeros((TAB_ROWS, D), dtype=np.float16)
    tab[_node_to_row(np.arange(N_NODES))] = x16
    return tab


# --------------------------------------------------------------------------
# launch 1: P = x @ A   (distributed over node slabs)
# --------------------------------------------------------------------------

def _build_l1(nt):
    """xt: [128, nt*128] f16 (x-slab transposed), amat: [128, 8] f16
    -> pout: [128, nt*8] f32 (node k*128+p at [p, k*8:(k+1)*8])."""
    nc = bacc.Bacc(None)
    xt = nc.declare_dram_parameter("xt", [128, nt * 128], F16, isOutput=False)
    amat = nc.declare_dram_parameter("amat", [128, 8], F16, isOutput=False)
    pout = nc.declare_dram_parameter("pout", [128, nt * 8], F32, isOutput=True)

    CHUNK = 14
    with tile.TileContext(nc) as tc:
        with (
            tc.tile_pool(name="sb", bufs=1) as sb,
            tc.tile_pool(name="ps", bufs=2, space="PSUM") as ps,
        ):
            a_sb = sb.tile([128, 8], F16)
            nc.sync.dma_start(out=a_sb[:], in_=amat[:, :])
            xt_sb = sb.tile([128, nt * 128], F16)
            nc.sync.dma_start(out=xt_sb[:], in_=xt[:, :])
            p_sb = sb.tile([128, nt * 8], F32)
            for j0 in range(0, nt, CHUNK):
                j1 = min(j0 + CHUNK, nt)
                p_ps = ps.tile([128, CHUNK * 8], F32)
                for k in range(j0, j1):
                    nc.tensor.matmul(out=p_ps[:, (k - j0) * 8:(k - j0 + 1) * 8],
                                     lhsT=xt_sb[:, k * 128:(k + 1) * 128],
                                     rhs=a_sb[:], start=True, stop=True)
                nc.vector.tensor_copy(out=p_sb[:, j0 * 8:j1 * 8],
                                      in_=p_ps[:, :(j1 - j0) * 8])
            nc.sync.dma_start(out=pout[:, :], in_=p_sb[:])
    nc.compile()
    return nc


# --------------------------------------------------------------------------
# launch 2: the main edge-parallel kernel
# --------------------------------------------------------------------------

def _build_l2(G):
    SG = -(-G // K)
    nc = bacc.Bacc(None, dynamic_dma_scratch_size=SCRATCH,
                   num_swdge_queues=NQ)
    tab = nc.declare_dram_parameter("tab", [TAB_ROWS, D], F16, isOutput=False)
    idxc = nc.declare_dram_parameter(
        "idxc", [SG, 128, NCHUNK * (NIDXC // 16)], I16, isOutput=False)
    ps8 = nc.declare_dram_parameter("ps8", [G, 128, CG, 8], F16, isOutput=False)
    locd = nc.declare_dram_parameter("locd", [G, 128, CG], F16, isOutput=False)
    iotac = nc.declare_dram_parameter("iotac", [128, W], F16, isOutput=False)
    selc = nc.declare_dram_parameter("selc", [4, 512], F16, isOutput=False)
    out = nc.declare_dram_parameter(
        "out", [G, 128, 4 * SPG * W], F16, isOutput=True)

    bases = [B0, B1]
    NXC = K * SPG            # xg columns per chunk run

    with tile.TileContext(nc) as tc:
        with (
            tc.tile_pool(name="cst", bufs=1) as cst,
            tc.tile_pool(name="idx", bufs=3) as idxp,
            tc.tile_pool(name="met", bufs=4) as met,
            tc.tile_pool(name="gat", bufs=3) as gat,
            tc.tile_pool(name="mm", bufs=3) as mm,
            tc.tile_pool(name="epi", bufs=2) as epi,
            tc.tile_pool(name="outp", bufs=2) as outp,
            tc.tile_pool(name="ps", bufs=2, space="PSUM") as ps,
            tc.tile_pool(name="psr", bufs=2, space="PSUM") as psr,
            tc.tile_pool(name="psb", bufs=2, space="PSUM") as psb,
        ):
            iota_sb = cst.tile([128, W], F16)
            nc.sync.dma_start(out=iota_sb[:], in_=iotac[:, :])
            sel_sb = cst.tile([4, 512], F16)
            nc.sync.dma_start(out=sel_sb[:], in_=selc[:, :])
            eps_sb = cst.tile([4, 1], F32)
            nc.gpsimd.memset(eps_sb[:], 1e-4)

            for sg in range(SG):
                kk = min(K, G - sg * K)
                nidx = kk * SPG * TILE_E
                ixc = idxp.tile([128, NCHUNK * (NIDXC // 16)], I16, tag="ixc")
                nc.sync.dma_start(out=ixc[:], in_=idxc[sg, :, :])
                xg = gat.tile([128, NCHUNK * NXC, D], F16, tag="xg")
                for cc in range(NCHUNK):
                    # single_packet=False: the HW packet ceiling is 64 descs
                    # (= 1024 idxs); coalescing a bigger gather into one
                    # packet wedges the device
                    nc.gpsimd.dma_gather(
                        out_ap=xg[:, cc * NXC:cc * NXC + kk * SPG, :],
                        in_ap=tab[bases[cc]:, :],
                        idxs_ap=ixc[:, cc * (NIDXC // 16):
                                    cc * (NIDXC // 16) + nidx // 16],
                        num_idxs=nidx, num_idxs_reg=nidx, elem_size=D,
                        queue_num=cc % NQ, single_packet=False)

                for gl in range(kk):
                    g = sg * K + gl
                    pp = met.tile([128, CG, 8], F16, tag="pp")
                    lo = met.tile([128, CG], F16, tag="lo")
                    nc.sync.dma_start(out=pp[:], in_=ps8[g, :, :, :])
                    nc.sync.dma_start(out=lo[:], in_=locd[g, :, :])

                    # ---- scores: e = exp(-max(s, 0.2 s)), s = p_src + p_dst
                    s16 = mm.tile([128, CG, 4], F16, tag="s16")
                    nc.vector.tensor_tensor(out=s16[:], in0=pp[:, :, 0:4],
                                            in1=pp[:, :, 4:8],
                                            op=mybir.AluOpType.add)
                    lr16 = mm.tile([128, CG, 4], F16, tag="lr16")
                    nc.vector.scalar_tensor_tensor(
                        out=lr16[:], in0=s16[:], scalar=0.2, in1=s16[:],
                        op0=mybir.AluOpType.mult, op1=mybir.AluOpType.max)
                    e4 = mm.tile([128, CG, 4], F16, tag="e4")
                    nc.scalar.activation(out=e4[:], in_=lr16[:],
                                         func=mybir.ActivationFunctionType.Exp,
                                         scale=-1.0)

                    # ---- one-hot and per-head scaled one-hot (single op)
                    m0 = mm.tile([128, CG, W], F16, tag="m0")
                    nc.vector.tensor_tensor(
                        out=m0[:],
                        in0=lo[:, :, None].broadcast_to([128, CG, W]),
                        in1=iota_sb[:, None, :].broadcast_to([128, CG, W]),
                        op=mybir.AluOpType.is_equal)
                    mall = mm.tile([128, CG, 4, W], F16, tag="mall")
                    nc.vector.tensor_tensor(
                        out=mall[:],
                        in0=m0[:, :, None, :].broadcast_to([128, CG, 4, W]),
                        in1=e4[:, :, :, None].broadcast_to([128, CG, 4, W]),
                        op=mybir.AluOpType.mult)

                    # ---- segment sums; the NCHUNK columns of a supertile
                    # accumulate into one PSUM slot range
                    # NOTE: the cc-chain of one slot must stay adjacent —
                    # start=True clears has_written for the WHOLE bank, so
                    # interleaving other slots' start=True between a slot's
                    # start and its accumulate corrupts the sum
                    agg = ps.tile([128, SPG * 4 * W], F32, tag="agg")
                    rs = psr.tile([4, SPG * W], F32, tag="rs")
                    for s in range(SPG):
                        for cc in range(NCHUNK):
                            xcol = cc * NXC + gl * SPG + s
                            ccol = cc * SPG + s
                            nc.tensor.matmul(
                                out=agg[:, s * 4 * W:(s + 1) * 4 * W],
                                lhsT=xg[:, xcol, :],
                                rhs=mall[:, ccol, :, :],
                                start=(cc == 0), stop=(cc == NCHUNK - 1))
                            nc.tensor.matmul(
                                out=rs[:, s * W:(s + 1) * W],
                                lhsT=e4[:, ccol, :], rhs=m0[:, ccol, :],
                                start=(cc == 0), stop=(cc == NCHUNK - 1))

                    # ---- epilogue: out = (w/rowsum) * agg, with w folded
                    # into the broadcast selector. 1/rs via ACT ln+exp; the
                    # bias clamps pad slots (real rowsums >= ~0.14)
                    lnr = epi.tile([4, SPG * W], F32, tag="lnr")
                    nc.scalar.activation(out=lnr[:], in_=rs[:],
                                         func=mybir.ActivationFunctionType.Ln,
                                         bias=eps_sb[:])
                    rsi16 = epi.tile([4, SPG * W], F16, tag="rsi16")
                    nc.scalar.activation(out=rsi16[:], in_=lnr[:],
                                         func=mybir.ActivationFunctionType.Exp,
                                         scale=-1.0)
                    rinv4 = epi.tile([128, 4, SPG * W], F16, tag="rinv4")
                    for i in range(4):
                        bc = psb.tile([128, SPG * W], F32, tag="bc")
                        nc.tensor.matmul(out=bc[:],
                                         lhsT=sel_sb[:, i * 128:(i + 1) * 128],
                                         rhs=rsi16[:], start=True, stop=True)
                        nc.scalar.activation(
                            out=rinv4[:, i, :], in_=bc[:],
                            func=mybir.ActivationFunctionType.Copy)
                    agg4 = agg[:].rearrange("p (s h w) -> p h s w",
                                            s=SPG, h=4, w=W)
                    oh = outp.tile([128, 4, SPG * W], F16, tag="oh")
                    nc.vector.tensor_tensor(
                        out=oh[:].rearrange("p h (s w) -> p h s w",
                                            s=SPG, w=W),
                        in0=agg4,
                        in1=rinv4[:].rearrange("p h (s w) -> p h s w",
                                               s=SPG, w=W),
                        op=mybir.AluOpType.mult)
                    nc.sync.dma_start(out=out[g, :, :], in_=oh[:])
    nc.compile()
    return nc


# --------------------------------------------------------------------------
# entry point
# --------------------------------------------------------------------------

def kernel(x, w, attn, edge):
    x = np.asarray(x, dtype=np.float32)
    w = np.asarray(w, dtype=np.float32)
    attn = np.asarray(attn, dtype=np.float32)
    edge = np.asarray(edge)

    n_nodes, d = x.shape
    n_heads = w.shape[0]
    assert d == D and n_heads == 4 and n_nodes == N_NODES

    src = edge[0].astype(np.int64)
    dst = edge[1].astype(np.int64)

    amat = np.zeros((128, 8), dtype=np.float32)
    for i in range(n_heads):
        amat[:, i] = w[i, 0, :] * attn[i, :d, 0]
        amat[:, 4 + i] = w[i, 0, :] * attn[i, d:, 0]

    trace = bool(int(os.environ.get("GAT_TRACE", "0")))
    tkw = dict(trace=True, trace_cores=list(range(N_CORES))) if trace else {}

    def _run(nc, maps):
        try:
            return run_bass_kernel_spmd(nc, maps, list(range(N_CORES)), **tkw)
        except Exception:
            if not tkw:
                raise
            return run_bass_kernel_spmd(nc, maps, list(range(N_CORES)))

    # ---------------- launch 1: P = x @ A (node slabs)
    x16 = x.astype(np.float16)
    npc = n_nodes // N_CORES
    nt = (npc + 127) // 128
    nc1 = _build_l1(nt)
    amat16 = amat.astype(np.float16)
    in_maps1 = []
    for c in range(N_CORES):
        sl = x16[c * npc:(c + 1) * npc]
        if sl.shape[0] < nt * 128:
            sl = np.concatenate(
                [sl, np.zeros((nt * 128 - sl.shape[0], d), np.float16)])
        in_maps1.append({"xt": np.ascontiguousarray(sl.T), "amat": amat16})
    r1 = _run(nc1, in_maps1)
    ptab = np.concatenate(
        [r1.results[c]["pout"].reshape(128, nt, 8).transpose(1, 0, 2)
         .reshape(-1, 8)[:npc] for c in range(N_CORES)], axis=0)
    ptab = np.ascontiguousarray(ptab[:n_nodes])

    # ---------------- host layout prep
    prep = _prep_edges(src, dst, n_nodes, ptab)
    G = prep["G"]

    # ---------------- launch 2
    nc2 = _build_l2(G)
    tabv = _build_table(x16)
    iota_c = np.broadcast_to(np.arange(W, dtype=np.float16), (128, W)).copy()
    sel_c = np.zeros((4, 512), dtype=np.float16)
    for i in range(4):
        sel_c[i, i * 128:(i + 1) * 128] = w[i, 0, :].astype(np.float16)
    in_maps2 = []
    for c in range(N_CORES):
        in_maps2.append({
            "tab": tabv,
            "idxc": prep["idxc"][c], "ps8": prep["ps8"][c],
            "locd": prep["loc"][c],
            "iotac": iota_c, "selc": sel_c,
        })
    r2 = _run(nc2, in_maps2)
    LAST_RESULTS.clear()
    LAST_RESULTS.extend([r1, r2])

    # ---------------- unshard
    out_full = np.zeros((n_heads, n_nodes, d), dtype=np.float32)
    for c in range(N_CORES):
        slab = r2.results[c]["out"]   # [G, 128, 4*SPG*W]
        cm = prep["colmap"][c]        # [G*SPG*W] -> node or -1
        arr = slab.reshape(G, 128, 4, SPG * W).transpose(2, 0, 3, 1).reshape(
            n_heads, G * SPG * W, d)
        valid = cm >= 0
        out_full[:, cm[valid], :] = arr[:, valid, :].astype(np.float32)
    return out_full


if __name__ == "__main__":
    pass
